# revision 28
# baseline (speedup 1.0000x reference)
"""Trainium2 Bass kernel for nn_Encoder_4724464025749 (tree-GRU encoder).

Strategy
--------
Pure data parallelism: batch B=4096 is split across 8 NeuronCores (512
columns each).  Each core runs the full 127-node binary-tree recursion for
its batch shard with all tensors kept feature-major ([feature partitions,
batch columns]) so every matmul contracts over the partition dimension and
hidden states never leave SBUF.  Per core the 512 columns are split into
SPLITS independent tree walks, emitted interleaved in post-order, so the
Tile scheduler always has several independent nodes in flight.

Precision: the attention normalization a = s / (s0 + s1) makes the model
chaotic - matmul noise of 1e-4 (plain f32r) explodes to ~0.19 relative
error, so every hidden-state matmul runs as an exact 3-term f32r split:
W @ h = Whi @ hhi + Whi @ hlo + Wlo @ hhi with Whi/Wlo split to 11-bit
mantissa halves on the host and hhi/hlo split on device (bitwise-AND
truncation + subtract).  f32r operands are rounded to 11 explicit mantissa
bits by the PE, so the pre-split halves pass through exactly and the
product is fp32-accurate (~2^-22) at 3 cycles/row instead of fp32's 4
(and 1 cycle/row per pass vs fp32's effective 4 passes).  The x
projections use the same trick with a single stacked-K matmul.  Scores
and the attention-weight broadcasts stay in true fp32 (cheap, M=1/K=1).
The split work runs spread across DVE, GPSIMD and ACT so the PE stays
the bottleneck.
"""

import numpy as np

DEPTH = 7
H = 256
I = 32
O = 128
B = 4096
NCORES = 8
P = 128
HT = H // P          # feature tiles per vector
KSP = 3 * I + 2      # split x contraction: xhi | 1 | xlo | 1 | xhi
CH = 2               # nodes per x/mask DMA chunk
NCOL = B // NCORES   # batch columns per core
SPLITS = 2           # independent tree walks per core
import os as _os
BLOCK = int(_os.environ.get("K_BLOCK", "4"))   # post-order block level
LAG = int(_os.environ.get("K_LAG", "0"))       # walk-1 emission lag


def _post_order(depth, block=BLOCK):
    """Post-order walk, but subtrees rooted at `block` level are emitted
    internally in bottom-up level order (wider ready-set for the scheduler
    while keeping the DFS-bounded live set above the block level)."""
    order = []

    def rec(d, j):
        if d == block and depth - 1 > d:
            for dd in range(depth - 1, d - 1, -1):
                for jj in range(j << (dd - d), (j + 1) << (dd - d)):
                    order.append((dd, jj))
            return
        if d < depth - 1:
            rec(d + 1, 2 * j)
            rec(d + 1, 2 * j + 1)
        order.append((d, j))

    rec(0, 0)
    return order


def _gid(d, j):
    return 2 ** d - 1 + j


def _round11(x):
    """Round fp32 to 11 explicit mantissa bits (the f32r operand grid)."""
    x = np.ascontiguousarray(np.asarray(x, dtype=np.float32))
    b = x.view(np.uint32)
    r = ((b + np.uint32(0x800)) >> np.uint32(12)) << np.uint32(12)
    return r.view(np.float32)


_MODULE_CACHE = {}


def _build_module(depth=DEPTH, ncol=NCOL, use_bias=False, mode="f32",
                  num_devices=NCORES, splits=SPLITS, use_mask=True):
    key = (depth, ncol, use_bias, mode, num_devices, splits, use_mask)
    if key in _MODULE_CACHE:
        return _MODULE_CACHE[key]

    import concourse.mybir as mybir
    import concourse.tile as tile
    from concourse import bacc

    dt = mybir.dt
    ACT_F = mybir.ActivationFunctionType
    ALU = mybir.AluOpType
    rdt = dt.float32r     # storage dtype of everything feeding f32r matmuls

    nodes = 2 ** depth - 1
    order = _post_order(depth)
    nsub = ncol // splits                        # columns per tree walk

    nc = bacc.Bacc("TRN2", num_devices=num_devices, debug=False)

    xT_d = nc.dram_tensor("xT", [KSP, nodes, ncol], rdt, kind="ExternalInput").ap()
    if use_mask:
        mb_d = nc.dram_tensor("maskb", [P, nodes, ncol], rdt,
                              kind="ExternalInput").ap()
    wi_d = nc.dram_tensor("wi", [KSP, 3 * H], rdt, kind="ExternalInput").ap()
    WH = ["whr", "whz", "whn", "wa", "wms0", "wms1"]
    wh_d = {name + sfx: nc.dram_tensor(name + sfx, [P, HT, H], rdt,
                                       kind="ExternalInput").ap()
            for name in WH for sfx in ("hi", "lo")}
    wsc_d = nc.dram_tensor("wsc", [P, HT, 1], dt.float32,
                           kind="ExternalInput").ap()
    wout_d = nc.dram_tensor("wout", [P, HT, 2 * O], dt.float32,
                            kind="ExternalInput").ap()
    ones_d = nc.dram_tensor("ones1", [1, P], dt.float32,
                            kind="ExternalInput").ap()
    ones2_d = nc.dram_tensor("ones2", [2, P], rdt,
                             kind="ExternalInput").ap()
    bias_d = nc.dram_tensor("biases", [P, 9], dt.float32,
                            kind="ExternalInput").ap()
    out_d = nc.dram_tensor("out", [2, P, ncol], dt.float32,
                           kind="ExternalOutput").ap()

    with tile.TileContext(nc) as tc:
        with tc.tile_pool(name="wpool", bufs=1) as wpool, \
             tc.tile_pool(name="xpool", bufs=3 * splits) as xpool, \
             tc.tile_pool(name="mpool", bufs=2 * splits) as mpool, \
             tc.tile_pool(name="hpool", bufs=11 * splits) as hpool, \
             tc.tile_pool(name="vpool", bufs=15 * splits) as vpool, \
             tc.tile_pool(name="cpool", bufs=8 * splits) as cpool, \
             tc.tile_pool(name="spool", bufs=4) as spool, \
             tc.tile_pool(name="opool", bufs=2) as opool, \
             tc.tile_pool(name="ppool0", bufs=4, space="PSUM") as ppool0, \
             tc.tile_pool(name="ppool1", bufs=4, space="PSUM") as ppool1:

            # ---- load weights once ----
            def wtile(dram, shape, dtype):
                t = wpool.tile(shape, dtype, tag=dram.name, name="w_" + dram.name)
                nc.sync.dma_start(out=t[:], in_=dram[:])
                return t

            wi_t = wtile(wi_d, [KSP, 3 * H], rdt)
            wsc_t = wtile(wsc_d, [P, HT, 1], dt.float32)
            ones_t = wtile(ones_d, [1, P], dt.float32)
            ones2_t = wtile(ones2_d, [2, P], rdt)
            bias_t = wtile(bias_d, [P, 9], dt.float32)

            # chunked x / mask staging, per tree walk
            x_tiles = {}
            m_tiles = {}

            def get_chunk(w, t):
                c = t // CH
                if (w, c) not in x_tiles:
                    n0 = c * CH
                    n1 = min(n0 + CH, nodes)
                    c0, c1 = w * nsub, (w + 1) * nsub
                    xt = xpool.tile([KSP, CH, nsub], rdt, tag="xchunk",
                                    name="xchunk")
                    nc.sync.dma_start(out=xt[:, : n1 - n0, :],
                                      in_=xT_d[:, n0:n1, c0:c1])
                    if use_mask:
                        mt = mpool.tile([P, CH, nsub], rdt, tag="mchunk",
                                        name="mchunk")
                        nc.sync.dma_start(out=mt[:, : n1 - n0, :],
                                          in_=mb_d[:, n0:n1, c0:c1])
                    else:
                        mt = None
                    x_tiles[(w, c)] = xt
                    m_tiles[(w, c)] = mt
                return x_tiles[(w, c)], m_tiles[(w, c)], t - c * CH

            # leaves only need wi + their x chunk: prefetch the first chunks
            # ahead of the heavy hidden-weight DMAs so the PE starts early
            for _w in range(splits):
                for _c in range(2):
                    get_chunk(_w, _c * CH)
            wh_t = {k: wtile(d, [P, HT, H], rdt) for k, d in wh_d.items()}
            wout_t = wtile(wout_d, [P, HT, 2 * O], dt.float32)

            ppools = [ppool0, ppool1]

            def psum_tile(w=0):
                return ppools[w].tile([P, HT, nsub], dt.float32, tag="ps",
                                      name="ps")

            def work_tile(dtype=dt.float32):
                return vpool.tile([P, HT, nsub], dtype, tag="work", name="work")

            def split_tile():
                return cpool.tile([P, HT, nsub], rdt, tag="spl", name="spl")

            def split11(x_t, eng_hi, eng_lo):
                """Return (xhi, xlo) f32r halves: writes into float32r tiles
                are rounded to the 11-bit f32r grid by the engine, so a plain
                copy + subtract is an exact hi/lo split."""
                xhi = split_tile()
                eng_hi.tensor_copy(xhi[:], x_t[:])
                xlo = split_tile()
                eng_lo.tensor_sub(xlo[:], x_t[:], xhi[:])
                return xhi, xlo

            def mm3(ps, wname, rhi, rlo, mt, start, stop):
                """ps[:, mt] (+)= W.T @ r via exact 3-term f32r split (K=3H)."""
                whi = wh_t[wname + "hi"]
                wlo = wh_t[wname + "lo"]
                for term, (w_t, r_t) in enumerate(
                        ((whi, rhi), (whi, rlo), (wlo, rhi))):
                    for kt in range(HT):
                        nc.tensor.matmul(
                            ps[:, mt, :],
                            lhsT=w_t[:, kt, mt * P:(mt + 1) * P],
                            rhs=r_t[:, kt, :],
                            start=(start and term == 0 and kt == 0),
                            stop=(stop and term == 2 and kt == HT - 1),
                        )

            def mm_x(ps, mt, col0, xc, xi, start, stop):
                """ps[:, mt] (+)= wi[:, col0+mt*P : col0+(mt+1)*P].T @ x."""
                nc.tensor.matmul(
                    ps[:, mt, :],
                    lhsT=wi_t[:, col0 + mt * P: col0 + (mt + 1) * P],
                    rhs=xc[:, xi, :],
                    start=start,
                    stop=stop,
                )

            def act(out_ap, in_ap, func, bias=0.0):
                nc.scalar.activation(out_ap, in_ap, func, bias=bias)

            def emit_leaf(w, t, finish):
                xc, mc, xi = get_chunk(w, t)
                # z = sigmoid(Wiz x + bz) ; n = tanh(Win x + bn)
                psz = psum_tile(w)
                for mt in range(HT):
                    mm_x(psz, mt, H, xc, xi, True, True)
                z = work_tile()
                act(z[:], psz[:], ACT_F.Sigmoid)
                psn = psum_tile(w)
                for mt in range(HT):
                    mm_x(psn, mt, 2 * H, xc, xi, True, True)
                n = work_tile()
                act(n[:], psn[:], ACT_F.Tanh)
                yield
                # h = (1-z)*n * m = (n - z*n) * m
                t1 = work_tile()
                nc.vector.tensor_mul(t1[:], z[:], n[:])
                h = hpool.tile([P, HT, nsub], dt.float32, tag="h", name="h")
                if use_mask:
                    nc.gpsimd.tensor_sub(t1[:], n[:], t1[:])
                    mbc = mc[:, xi:xi + 1, :].to_broadcast((P, HT, nsub))
                    nc.vector.tensor_mul(h[:], t1[:], mbc)
                else:
                    nc.gpsimd.tensor_sub(h[:], n[:], t1[:])
                finish(h)

            def emit_internal(w, t, d, hl, hr, finish):
                xc, mc, xi = get_chunk(w, t)

                # ---- split children into 11-bit hi/lo halves ----
                chl = split11(hl, nc.gpsimd, nc.vector)   # (hi, lo) of left
                chr_ = split11(hr, nc.gpsimd, nc.vector)
                kids = (chl, chr_)
                raw = (hl, hr)

                # ---- r_k = sigmoid(xi_r + Whr c_k + b_r) ; s = sum r_k*c_k
                r = []
                for k in range(2):
                    psr = psum_tile(w)
                    for mt in range(HT):
                        mm_x(psr, mt, 0, xc, xi, True, False)
                        mm3(psr, "whr", kids[k][0], kids[k][1], mt, False, True)
                    rk = work_tile()
                    act(rk[:], psr[:], ACT_F.Sigmoid)
                    r.append(rk)
                yield
                t0 = work_tile()
                nc.vector.tensor_mul(t0[:], r[0][:], hl[:])
                t3 = work_tile()
                nc.gpsimd.tensor_mul(t3[:], r[1][:], hr[:])
                s = work_tile()
                nc.vector.tensor_add(s[:], t0[:], t3[:])
                shi = split_tile()
                nc.gpsimd.tensor_add(shi[:], t0[:], t3[:])  # f32r out = hi
                slo = split_tile()
                nc.vector.tensor_sub(slo[:], s[:], shi[:])
                shl = (shi, slo)

                # ---- attention: ms_k = tanh(Wms_k c_k + b_k) ----
                ms = []
                for k in range(2):
                    psm = psum_tile(w)
                    for mt in range(HT):
                        mm3(psm, "wms%d" % k, kids[k][0], kids[k][1], mt,
                            True, True)
                    mk = work_tile()
                    if use_bias:
                        for mt in range(HT):
                            act(mk[:, mt, :], psm[:, mt, :], ACT_F.Tanh,
                                bias=bias_t[:, 2 * k + mt: 2 * k + mt + 1])
                    else:
                        act(mk[:], psm[:], ACT_F.Tanh)
                    ms.append(mk)

                yield
                # ---- scores s_k = w . ms_k (+ w_b), fp32 exact ----
                pss = psum_tile(w)
                for k in range(2):
                    for kt in range(HT):
                        nc.tensor.matmul(
                            pss[0:1, k, :],
                            lhsT=wsc_t[:, kt, :],
                            rhs=ms[k][:, kt, :],
                            start=(kt == 0),
                            stop=(kt == HT - 1),
                        )
                sc = spool.tile([1, 2, nsub], dt.float32, tag="sc", name="sc")
                act(sc[:], pss[0:1, :, :], ACT_F.Identity)
                if use_bias:
                    nc.vector.tensor_scalar(sc[:], sc[:],
                                            bias_t[0:1, 8:9], None, ALU.add)
                s0, s1 = sc[:, 0, :], sc[:, 1, :]
                den = spool.tile([1, nsub], dt.float32, tag="den", name="den")
                nc.vector.tensor_add(den[:], s0, s1)
                rec = spool.tile([1, nsub], dt.float32, tag="rec", name="rec")
                nc.vector.reciprocal_approx_fast(rec[:], den[:])
                a0 = spool.tile([1, nsub], dt.float32, tag="a01", name="a0")
                nc.vector.tensor_mul(a0[:], s0, rec[:])

                yield
                # broadcast a0 across partitions via K=1 ones matmul (fp32);
                # g = a0*c0 + (1-a0)*c1 = c1 + a0*(c0 - c1)
                psb = psum_tile(w)
                nc.tensor.matmul(psb[:, 0, :], lhsT=ones_t[:],
                                 rhs=a0[:], start=True, stop=True)
                dlr = work_tile()
                nc.gpsimd.tensor_sub(dlr[:], hl[:], hr[:])
                gt = work_tile()
                nc.vector.tensor_mul(
                    gt[:], dlr[:], psb[:, 0:1, :].to_broadcast((P, HT, nsub)))
                g = work_tile()
                nc.vector.tensor_add(g[:], gt[:], hr[:])
                ghi = split_tile()
                nc.gpsimd.tensor_add(ghi[:], gt[:], hr[:])  # f32r out = hi
                glo = split_tile()
                nc.vector.tensor_sub(glo[:], g[:], ghi[:])
                ghl = (ghi, glo)
                yield
                psc = psum_tile(w)
                for mt in range(HT):
                    mm3(psc, "wa", ghl[0], ghl[1], mt, True, True)
                cs = work_tile()
                cshi = split_tile()
                if use_bias:
                    for mt in range(HT):
                        act(cs[:, mt, :], psc[:, mt, :], ACT_F.Tanh,
                            bias=bias_t[:, 4 + mt: 5 + mt])
                        act(cshi[:, mt, :], psc[:, mt, :], ACT_F.Tanh,
                            bias=bias_t[:, 4 + mt: 5 + mt])
                else:
                    act(cs[:], psc[:], ACT_F.Tanh)
                    act(cshi[:], psc[:], ACT_F.Tanh)  # f32r out = hi
                cslo = split_tile()
                nc.vector.tensor_sub(cslo[:], cs[:], cshi[:])
                cshl = (cshi, cslo)
                yield
                # ---- z, n, h ----
                psz = psum_tile(w)
                for mt in range(HT):
                    mm_x(psz, mt, H, xc, xi, True, False)
                    mm3(psz, "whz", cshl[0], cshl[1], mt, False, True)
                z = work_tile()
                act(z[:], psz[:], ACT_F.Sigmoid)
                psn = psum_tile(w)
                for mt in range(HT):
                    mm_x(psn, mt, 2 * H, xc, xi, True, False)
                    mm3(psn, "whn", shl[0], shl[1], mt, False, True)
                n = work_tile()
                act(n[:], psn[:], ACT_F.Tanh)
                yield
                # h = (n + z*(cs - n)) * m
                t4 = work_tile()
                nc.gpsimd.tensor_sub(t4[:], cs[:], n[:])
                nc.vector.tensor_mul(t4[:], z[:], t4[:])
                h = hpool.tile([P, HT, nsub], dt.float32, tag="h", name="h")
                if use_mask:
                    nc.vector.tensor_add(t4[:], n[:], t4[:])
                    mbc = mc[:, xi:xi + 1, :].to_broadcast((P, HT, nsub))
                    nc.vector.tensor_mul(h[:], t4[:], mbc)
                else:
                    nc.vector.tensor_add(h[:], n[:], t4[:])
                finish(h)

            # ---- software-pipelined tree walk ----
            # Each node is a generator with yields at dependency boundaries
            # (psum-group / activation / elementwise-chain edges).  A driver
            # advances all in-flight nodes round-robin, one stage per tick,
            # so every engine's program-order queue interleaves independent
            # nodes and the per-node serial chains overlap with other nodes'
            # matmul groups instead of head-of-line blocking them.
            hmaps = [{} for _ in range(splits)]
            roots = [None] * splits
            n_nodes = len(order)

            def node_gen(w, t):
                d, j = order[t]
                hmap = hmaps[w]

                def finish(h):
                    hmap[(d, j)] = h

                if d == depth - 1:
                    return emit_leaf(w, t, finish)
                hl = hmap.pop((d + 1, 2 * j))
                hr = hmap.pop((d + 1, 2 * j + 1))
                return emit_internal(w, t, d, hl, hr, finish)

            WIN = int(_os.environ.get("K_WIN", "2"))    # internals in flight
            WINL = int(_os.environ.get("K_WINL", "0"))  # extra leaf lookahead
            active = []                                 # [walk, is_leaf, gen]
            next_i = [0] * splits
            while active or any(next_i[w] < n_nodes for w in range(splits)):
                for w in range(splits):
                    while next_i[w] < n_nodes:
                        if (w > 0 and next_i[w - 1] < n_nodes and
                                next_i[w] + LAG > next_i[w - 1]):
                            break
                        t = next_i[w]
                        d, j = order[t]
                        leaf = d == depth - 1
                        n_int = sum(1 for a in active
                                    if a[0] == w and not a[1])
                        n_all = sum(1 for a in active if a[0] == w)
                        if not leaf and (
                                (d + 1, 2 * j) not in hmaps[w] or
                                (d + 1, 2 * j + 1) not in hmaps[w]):
                            break
                        if leaf:
                            if n_all >= WIN + WINL:
                                break
                        elif n_int >= WIN or n_all >= WIN + WINL:
                            break
                        active.append([w, leaf, node_gen(w, t)])
                        next_i[w] += 1
                for a in list(active):
                    try:
                        next(a[2])
                    except StopIteration:
                        active.remove(a)
            for w in range(splits):
                roots[w] = hmaps[w][(0, 0)]

            # ---- output heads (fp32) ----
            for w in range(splits):
                root = roots[w]
                c0, c1 = w * nsub, (w + 1) * nsub
                pso = psum_tile(w)
                for oi in range(2):
                    for kt in range(HT):
                        nc.tensor.matmul(
                            pso[:, oi, :],
                            lhsT=wout_t[:, kt, oi * O:(oi + 1) * O],
                            rhs=root[:, kt, :],
                            start=(kt == 0),
                            stop=(kt == HT - 1),
                        )
                for oi in range(2):
                    ot = opool.tile([P, nsub], dt.float32, tag="osb", name="osb")
                    if use_bias:
                        act(ot[:], pso[:, oi, :], ACT_F.Identity,
                            bias=bias_t[:, 6 + oi: 7 + oi])
                    else:
                        act(ot[:], pso[:, oi, :], ACT_F.Identity)
                    nc.sync.dma_start(out=out_d[oi, :, c0:c1], in_=ot[:])

    nc.compile()
    _MODULE_CACHE[key] = nc
    return nc


def _to_f32(arr):
    return np.ascontiguousarray(np.asarray(arr, dtype=np.float32))


def _pack_weights(inputs, mode="f32"):
    """Host-side packing of weights into device lhsT layouts."""
    f32 = lambda k: np.asarray(inputs[k], dtype=np.float32)

    def lhsT_h(w):  # [H, H] torch-layout -> [P, HT, H]
        return w.T.reshape(HT, P, w.shape[0]).transpose(1, 0, 2)

    wir_w, wiz_w, win_w = f32("wir_w"), f32("wiz_w"), f32("win_w")
    br = f32("wir_b") + f32("whr_b")
    bz = f32("wiz_b") + f32("whz_b")
    bn = f32("win_b") + f32("whn_b")
    wcat = np.concatenate([wir_w, wiz_w, win_w], axis=0)      # [3H, I]
    bcat = np.concatenate([br, bz, bn])                       # [3H]
    # exact split-K layout: rows = xhi*Whi | 1*bhi | xlo*Whi | 1*blo | xhi*Wlo
    w_hi = _round11(wcat)
    w_lo = wcat - w_hi
    b_hi = _round11(bcat)
    b_lo = bcat - b_hi
    wi = np.concatenate([
        w_hi.T, b_hi[None, :], w_hi.T, b_lo[None, :], w_lo.T], axis=0)

    wms = f32("wms_w")                                        # [2, H, H]
    wsc = f32("w_w").T.reshape(HT, P, 1).transpose(1, 0, 2)   # [P, HT, 1]
    wout = lhsT_h(np.concatenate([f32("mu_w"), f32("lv_w")], axis=0))

    biases = np.zeros((P, 9), dtype=np.float32)
    wms_b = f32("wms_b")                                      # [2, H]
    for k in range(2):
        for mt in range(HT):
            biases[:, 2 * k + mt] = wms_b[k, mt * P:(mt + 1) * P]
    wa_b = f32("wa_b")
    for mt in range(HT):
        biases[:, 4 + mt] = wa_b[mt * P:(mt + 1) * P]
    biases[:, 6] = f32("mu_b")
    biases[:, 7] = f32("lv_b")
    biases[:, 8] = float(np.asarray(inputs["w_b"]).reshape(-1)[0])

    packed = {
        "wi": _to_f32(wi),
        "wsc": _to_f32(wsc),
        "wout": _to_f32(wout),
        "ones1": _to_f32(np.ones((1, P))),
        "ones2": _to_f32(np.ones((2, P))),
        "biases": biases,
    }
    hw_mats = {"whr": f32("whr_w"), "whz": f32("whz_w"), "whn": f32("whn_w"),
               "wa": f32("wa_w"), "wms0": wms[0], "wms1": wms[1]}
    for name, w in hw_mats.items():
        whi = _round11(w)
        wlo = _round11(w - whi)
        packed[name + "hi"] = _to_f32(lhsT_h(whi))
        packed[name + "lo"] = _to_f32(lhsT_h(wlo))

    use_bias = any(
        float(np.abs(np.asarray(inputs[k])).max()) != 0.0
        for k in ("wms_b", "wa_b", "w_b", "mu_b", "lv_b")
    )
    return packed, use_bias


def _pack_percore(targets, mask, use_mask, depth=DEPTH, ncol=NCOL,
                  ncores=NCORES):
    order = _post_order(depth)
    perm = np.array([_gid(d, j) for (d, j) in order])
    nodes = len(order)
    bsz = targets.shape[1]

    tg = np.asarray(targets, dtype=np.float32)[:, :, 0, :]    # [nodes, B, I]
    xall = tg.transpose(2, 0, 1)[:, perm, :]                  # [I, nodes, B]
    x_hi = _round11(xall)
    x_lo = xall - x_hi
    ones = np.ones((1, nodes, bsz), np.float32)
    xaug = np.concatenate([x_hi, ones, x_lo, ones, x_hi], axis=0)  # [KSP,...]

    per_core = []
    for c in range(ncores):
        cols = slice(c * ncol, (c + 1) * ncol)
        pc = {"xT": np.ascontiguousarray(xaug[:, :, cols])}
        if use_mask:
            mpost = np.asarray(mask, dtype=np.float32)[perm]  # [nodes, B]
            pc["maskb"] = np.ascontiguousarray(
                np.broadcast_to(mpost[None, :, cols], (P, nodes, ncol)))
        per_core.append(pc)
    return per_core


def kernel(**inputs):
    import sys
    try:
        import concourse.bass  # noqa: F401
    except ImportError:
        sys.path.insert(0, "/opt/trn_rl_repo")

    try:
        import antenv.axon_hooks  # noqa: F401
    except ImportError:
        # absent in trimmed containers; run_bass_kernel_spmd imports it
        # unconditionally when BASS_TRACE is set — stub the no-hook path
        import types
        _m = types.ModuleType("antenv.axon_hooks")
        _m.get_axon_ntff_profile_hook = lambda: None
        sys.modules["antenv.axon_hooks"] = _m

    from concourse import bass_utils

    packed, use_bias = _pack_weights(inputs)
    use_mask = bool(np.any(np.asarray(inputs["mask"]) != 1.0))
    nc = _build_module(use_bias=use_bias, use_mask=use_mask)
    per_core = _pack_percore(inputs["targets"], inputs["mask"], use_mask)

    in_maps = [{**pc, **packed} for pc in per_core]
    res = bass_utils.run_bass_kernel_spmd(
        nc, in_maps, core_ids=list(range(NCORES)))

    mu = np.empty((B, 1, O), dtype=np.float32)
    lv = np.empty((B, 1, O), dtype=np.float32)
    for c in range(NCORES):
        out = res.results[c]["out"]                          # [2, P, ncol]
        cols = slice(c * NCOL, (c + 1) * NCOL)
        mu[cols, 0, :] = out[0].T
        lv[cols, 0, :] = out[1].T
    return mu, lv


# revision 38
# speedup vs baseline: 1.0084x; 1.0084x over previous
"""Trainium2 Bass kernel for nn_Encoder_4724464025749 (tree-GRU encoder).

Strategy
--------
Pure data parallelism: batch B=4096 is split across 8 NeuronCores (512
columns each).  Each core runs the full 127-node binary-tree recursion for
its batch shard with all tensors kept feature-major ([feature partitions,
batch columns]) so every matmul contracts over the partition dimension and
hidden states never leave SBUF.  Per core the 512 columns are split into
SPLITS independent tree walks, emitted interleaved in post-order, so the
Tile scheduler always has several independent nodes in flight.

Precision: the attention normalization a = s / (s0 + s1) makes the model
chaotic - matmul noise of 1e-4 (plain f32r) explodes to ~0.19 relative
error, so every hidden-state matmul runs as an exact 3-term f32r split:
W @ h = Whi @ hhi + Whi @ hlo + Wlo @ hhi with Whi/Wlo split to 11-bit
mantissa halves on the host and hhi/hlo split on device (bitwise-AND
truncation + subtract).  f32r operands are rounded to 11 explicit mantissa
bits by the PE, so the pre-split halves pass through exactly and the
product is fp32-accurate (~2^-22) at 3 cycles/row instead of fp32's 4
(and 1 cycle/row per pass vs fp32's effective 4 passes).  The x
projections use the same trick with a single stacked-K matmul.  Scores
and the attention-weight broadcasts stay in true fp32 (cheap, M=1/K=1).
The split work runs spread across DVE, GPSIMD and ACT so the PE stays
the bottleneck.
"""

import numpy as np

DEPTH = 7
H = 256
I = 32
O = 128
B = 4096
NCORES = 8
P = 128
HT = H // P          # feature tiles per vector
KSP = 3 * I + 2      # split x contraction: xhi | 1 | xlo | 1 | xhi
CH = 4               # nodes per x/mask DMA chunk
NCOL = B // NCORES   # batch columns per core
SPLITS = 2           # independent tree walks per core
import os as _os
BLOCK = int(_os.environ.get("K_BLOCK", "4"))   # post-order block level
LAG = int(_os.environ.get("K_LAG", "0"))       # walk-1 emission lag


def _post_order(depth, block=BLOCK):
    """Post-order walk, but subtrees rooted at `block` level are emitted
    internally in bottom-up level order (wider ready-set for the scheduler
    while keeping the DFS-bounded live set above the block level)."""
    order = []

    def rec(d, j):
        if d == block and depth - 1 > d:
            for dd in range(depth - 1, d - 1, -1):
                for jj in range(j << (dd - d), (j + 1) << (dd - d)):
                    order.append((dd, jj))
            return
        if d < depth - 1:
            rec(d + 1, 2 * j)
            rec(d + 1, 2 * j + 1)
        order.append((d, j))

    rec(0, 0)
    return order


def _gid(d, j):
    return 2 ** d - 1 + j


def _round11(x):
    """Round fp32 to 11 explicit mantissa bits (the f32r operand grid)."""
    x = np.ascontiguousarray(np.asarray(x, dtype=np.float32))
    b = x.view(np.uint32)
    r = ((b + np.uint32(0x800)) >> np.uint32(12)) << np.uint32(12)
    return r.view(np.float32)


_MODULE_CACHE = {}


def _build_module(depth=DEPTH, ncol=NCOL, use_bias=False, mode="f32",
                  num_devices=NCORES, splits=SPLITS, use_mask=True):
    key = (depth, ncol, use_bias, mode, num_devices, splits, use_mask)
    if key in _MODULE_CACHE:
        return _MODULE_CACHE[key]

    import concourse.mybir as mybir
    import concourse.tile as tile
    from concourse import bacc

    dt = mybir.dt
    ACT_F = mybir.ActivationFunctionType
    ALU = mybir.AluOpType
    rdt = dt.float32r     # storage dtype of everything feeding f32r matmuls

    nodes = 2 ** depth - 1
    order = _post_order(depth)
    nsub = ncol // splits                        # columns per tree walk

    nc = bacc.Bacc("TRN2", num_devices=num_devices, debug=False)

    xT_d = nc.dram_tensor("xT", [KSP, nodes, ncol], rdt, kind="ExternalInput").ap()
    if use_mask:
        mb_d = nc.dram_tensor("maskb", [P, nodes, ncol], rdt,
                              kind="ExternalInput").ap()
    wi_d = nc.dram_tensor("wi", [KSP, 3 * H], rdt, kind="ExternalInput").ap()
    WH = ["whr", "wms0", "wms1", "wa", "whz", "whn"]
    wh_d = {name + sfx: nc.dram_tensor(name + sfx, [P, HT, H], rdt,
                                       kind="ExternalInput").ap()
            for name in WH for sfx in ("hi", "lo")}
    wsc_d = nc.dram_tensor("wsc", [P, HT, 1], dt.float32,
                           kind="ExternalInput").ap()
    wout_d = nc.dram_tensor("wout", [P, HT, 2 * O], dt.float32,
                            kind="ExternalInput").ap()
    ones_d = nc.dram_tensor("ones1", [1, P], dt.float32,
                            kind="ExternalInput").ap()
    ones2_d = nc.dram_tensor("ones2", [2, P], rdt,
                             kind="ExternalInput").ap()
    bias_d = nc.dram_tensor("biases", [P, 9], dt.float32,
                            kind="ExternalInput").ap()
    out_d = nc.dram_tensor("out", [2, P, ncol], dt.float32,
                           kind="ExternalOutput").ap()

    with tile.TileContext(nc) as tc:
        with tc.tile_pool(name="wpool", bufs=1) as wpool, \
             tc.tile_pool(name="xpool", bufs=2 * splits) as xpool, \
             tc.tile_pool(name="mpool", bufs=2 * splits) as mpool, \
             tc.tile_pool(name="hpool", bufs=11 * splits) as hpool, \
             tc.tile_pool(name="vpool", bufs=15 * splits) as vpool, \
             tc.tile_pool(name="cpool", bufs=8 * splits) as cpool, \
             tc.tile_pool(name="spool", bufs=4) as spool, \
             tc.tile_pool(name="opool", bufs=2) as opool, \
             tc.tile_pool(name="ppool0", bufs=4, space="PSUM") as ppool0, \
             tc.tile_pool(name="ppool1", bufs=4, space="PSUM") as ppool1:

            # ---- load weights once ----
            def wtile(dram, shape, dtype):
                t = wpool.tile(shape, dtype, tag=dram.name, name="w_" + dram.name)
                nc.sync.dma_start(out=t[:], in_=dram[:])
                return t

            wi_t = wtile(wi_d, [KSP, 3 * H], rdt)
            wsc_t = wtile(wsc_d, [P, HT, 1], dt.float32)
            ones_t = wtile(ones_d, [1, P], dt.float32)
            ones2_t = wtile(ones2_d, [2, P], rdt)
            bias_t = wtile(bias_d, [P, 9], dt.float32)

            # chunked x / mask staging, per tree walk
            x_tiles = {}
            m_tiles = {}

            def get_chunk(w, t):
                c = t // CH
                if (w, c) not in x_tiles:
                    n0 = c * CH
                    n1 = min(n0 + CH, nodes)
                    c0, c1 = w * nsub, (w + 1) * nsub
                    xt = xpool.tile([KSP, CH, nsub], rdt, tag="xchunk",
                                    name="xchunk")
                    nc.sync.dma_start(out=xt[:, : n1 - n0, :],
                                      in_=xT_d[:, n0:n1, c0:c1])
                    if use_mask:
                        mt = mpool.tile([P, CH, nsub], rdt, tag="mchunk",
                                        name="mchunk")
                        nc.sync.dma_start(out=mt[:, : n1 - n0, :],
                                          in_=mb_d[:, n0:n1, c0:c1])
                    else:
                        mt = None
                    x_tiles[(w, c)] = xt
                    m_tiles[(w, c)] = mt
                return x_tiles[(w, c)], m_tiles[(w, c)], t - c * CH

            # leaves only need wi + their x chunk: prefetch the first chunks
            # ahead of the heavy hidden-weight DMAs so the PE starts early
            for _w in range(splits):
                for _c in range(2):
                    get_chunk(_w, _c * CH)
            wh_t = {k: wtile(d, [P, HT, H], rdt) for k, d in wh_d.items()}
            wout_t = wtile(wout_d, [P, HT, 2 * O], dt.float32)

            ppools = [ppool0, ppool1]

            def psum_tile(w=0):
                return ppools[w].tile([P, HT, nsub], dt.float32, tag="ps",
                                      name="ps")

            def work_tile(dtype=dt.float32):
                return vpool.tile([P, HT, nsub], dtype, tag="work", name="work")

            def split_tile():
                return cpool.tile([P, HT, nsub], rdt, tag="spl", name="spl")

            def split11(x_t, eng_hi, eng_lo):
                """Return (xhi, xlo) f32r halves: writes into float32r tiles
                are rounded to the 11-bit f32r grid by the engine, so a plain
                copy + subtract is an exact hi/lo split."""
                xhi = split_tile()
                eng_hi.tensor_copy(xhi[:], x_t[:])
                xlo = split_tile()
                eng_lo.tensor_sub(xlo[:], x_t[:], xhi[:])
                return xhi, xlo

            def mm3(ps, wname, rhi, rlo, mt, start, stop):
                """ps[:, mt] (+)= W.T @ r via exact 3-term f32r split (K=3H)."""
                whi = wh_t[wname + "hi"]
                wlo = wh_t[wname + "lo"]
                for term, (w_t, r_t) in enumerate(
                        ((whi, rhi), (whi, rlo), (wlo, rhi))):
                    for kt in range(HT):
                        nc.tensor.matmul(
                            ps[:, mt, :],
                            lhsT=w_t[:, kt, mt * P:(mt + 1) * P],
                            rhs=r_t[:, kt, :],
                            start=(start and term == 0 and kt == 0),
                            stop=(stop and term == 2 and kt == HT - 1),
                        )

            def mm_x(ps, mt, col0, xc, xi, start, stop):
                """ps[:, mt] (+)= wi[:, col0+mt*P : col0+(mt+1)*P].T @ x."""
                nc.tensor.matmul(
                    ps[:, mt, :],
                    lhsT=wi_t[:, col0 + mt * P: col0 + (mt + 1) * P],
                    rhs=xc[:, xi, :],
                    start=start,
                    stop=stop,
                )

            def act(out_ap, in_ap, func, bias=0.0):
                nc.scalar.activation(out_ap, in_ap, func, bias=bias)

            def emit_leaf(w, t, finish):
                xc, mc, xi = get_chunk(w, t)
                # zc = 1 - sigmoid(Wiz x) = sigmoid(-(Wiz x)) via ACT scale=-1
                psz = psum_tile(w)
                for mt in range(HT):
                    mm_x(psz, mt, H, xc, xi, True, True)
                zc = work_tile()
                nc.scalar.activation(zc[:], psz[:], ACT_F.Sigmoid, scale=-1.0)
                psn = psum_tile(w)
                for mt in range(HT):
                    mm_x(psn, mt, 2 * H, xc, xi, True, True)
                n = work_tile()
                act(n[:], psn[:], ACT_F.Tanh)
                yield
                # h = (1-z)*n * m
                h = hpool.tile([P, HT, nsub], dt.float32, tag="h", name="h")
                if use_mask:
                    t1 = work_tile()
                    nc.vector.tensor_mul(t1[:], zc[:], n[:])
                    mbc = mc[:, xi:xi + 1, :].to_broadcast((P, HT, nsub))
                    nc.vector.tensor_mul(h[:], t1[:], mbc)
                else:
                    nc.vector.tensor_mul(h[:], zc[:], n[:])
                finish(h)

            def emit_internal(w, t, d, hl, hr, finish):
                xc, mc, xi = get_chunk(w, t)

                # ---- split children into 11-bit hi/lo halves ----
                chl = split11(hl, nc.gpsimd, nc.vector)   # (hi, lo) of left
                chr_ = split11(hr, nc.gpsimd, nc.vector)
                kids = (chl, chr_)
                raw = (hl, hr)

                # ---- r_k = sigmoid(xi_r + Whr c_k + b_r) ; s = sum r_k*c_k
                r = []
                for k in range(2):
                    psr = psum_tile(w)
                    for mt in range(HT):
                        mm_x(psr, mt, 0, xc, xi, True, False)
                        mm3(psr, "whr", kids[k][0], kids[k][1], mt, False, True)
                    rk = work_tile()
                    act(rk[:], psr[:], ACT_F.Sigmoid)
                    r.append(rk)
                yield
                t0 = work_tile()
                nc.vector.tensor_mul(t0[:], r[0][:], hl[:])
                t3 = work_tile()
                nc.gpsimd.tensor_mul(t3[:], r[1][:], hr[:])
                s = work_tile()
                nc.vector.tensor_add(s[:], t0[:], t3[:])
                shi = split_tile()
                nc.gpsimd.tensor_add(shi[:], t0[:], t3[:])  # f32r out = hi
                slo = split_tile()
                nc.vector.tensor_sub(slo[:], s[:], shi[:])
                shl = (shi, slo)

                # ---- attention: ms_k = tanh(Wms_k c_k + b_k) ----
                ms = []
                for k in range(2):
                    psm = psum_tile(w)
                    for mt in range(HT):
                        mm3(psm, "wms%d" % k, kids[k][0], kids[k][1], mt,
                            True, True)
                    mk = work_tile()
                    if use_bias:
                        for mt in range(HT):
                            act(mk[:, mt, :], psm[:, mt, :], ACT_F.Tanh,
                                bias=bias_t[:, 2 * k + mt: 2 * k + mt + 1])
                    else:
                        act(mk[:], psm[:], ACT_F.Tanh)
                    ms.append(mk)

                yield
                # ---- scores s_k = w . ms_k (+ w_b), fp32 exact ----
                pss = psum_tile(w)
                for k in range(2):
                    for kt in range(HT):
                        nc.tensor.matmul(
                            pss[0:1, k, :],
                            lhsT=wsc_t[:, kt, :],
                            rhs=ms[k][:, kt, :],
                            start=(kt == 0),
                            stop=(kt == HT - 1),
                        )
                sc = spool.tile([1, 2, nsub], dt.float32, tag="sc", name="sc")
                act(sc[:], pss[0:1, :, :], ACT_F.Identity)
                if use_bias:
                    nc.vector.tensor_scalar(sc[:], sc[:],
                                            bias_t[0:1, 8:9], None, ALU.add)
                s0, s1 = sc[:, 0, :], sc[:, 1, :]
                den = spool.tile([1, nsub], dt.float32, tag="den", name="den")
                nc.vector.tensor_add(den[:], s0, s1)
                rec = spool.tile([1, nsub], dt.float32, tag="rec", name="rec")
                nc.vector.reciprocal_approx_fast(rec[:], den[:])
                a0 = spool.tile([1, nsub], dt.float32, tag="a01", name="a0")
                nc.vector.tensor_mul(a0[:], s0, rec[:])

                yield
                # broadcast a0 across partitions via K=1 ones matmul (fp32);
                # g = a0*c0 + (1-a0)*c1 = c1 + a0*(c0 - c1)
                psb = psum_tile(w)
                nc.tensor.matmul(psb[:, 0, :], lhsT=ones_t[:],
                                 rhs=a0[:], start=True, stop=True)
                dlr = work_tile()
                nc.gpsimd.tensor_sub(dlr[:], hl[:], hr[:])
                gt = work_tile()
                nc.vector.tensor_mul(
                    gt[:], dlr[:], psb[:, 0:1, :].to_broadcast((P, HT, nsub)))
                g = work_tile()
                nc.vector.tensor_add(g[:], gt[:], hr[:])
                ghi = split_tile()
                nc.gpsimd.tensor_add(ghi[:], gt[:], hr[:])  # f32r out = hi
                glo = split_tile()
                nc.vector.tensor_sub(glo[:], g[:], ghi[:])
                ghl = (ghi, glo)
                yield
                psc = psum_tile(w)
                for mt in range(HT):
                    mm3(psc, "wa", ghl[0], ghl[1], mt, True, True)
                cs = work_tile()
                cshi = split_tile()
                if use_bias:
                    for mt in range(HT):
                        act(cs[:, mt, :], psc[:, mt, :], ACT_F.Tanh,
                            bias=bias_t[:, 4 + mt: 5 + mt])
                        act(cshi[:, mt, :], psc[:, mt, :], ACT_F.Tanh,
                            bias=bias_t[:, 4 + mt: 5 + mt])
                else:
                    act(cs[:], psc[:], ACT_F.Tanh)
                    act(cshi[:], psc[:], ACT_F.Tanh)  # f32r out = hi
                cslo = split_tile()
                nc.vector.tensor_sub(cslo[:], cs[:], cshi[:])
                cshl = (cshi, cslo)
                yield
                # ---- z, n, h ----
                psz = psum_tile(w)
                for mt in range(HT):
                    mm_x(psz, mt, H, xc, xi, True, False)
                    mm3(psz, "whz", cshl[0], cshl[1], mt, False, True)
                z = work_tile()
                act(z[:], psz[:], ACT_F.Sigmoid)
                psn = psum_tile(w)
                for mt in range(HT):
                    mm_x(psn, mt, 2 * H, xc, xi, True, False)
                    mm3(psn, "whn", shl[0], shl[1], mt, False, True)
                n = work_tile()
                act(n[:], psn[:], ACT_F.Tanh)
                yield
                # h = (n + z*(cs - n)) * m
                t4 = work_tile()
                nc.gpsimd.tensor_sub(t4[:], cs[:], n[:])
                nc.vector.tensor_mul(t4[:], z[:], t4[:])
                h = hpool.tile([P, HT, nsub], dt.float32, tag="h", name="h")
                if use_mask:
                    nc.vector.tensor_add(t4[:], n[:], t4[:])
                    mbc = mc[:, xi:xi + 1, :].to_broadcast((P, HT, nsub))
                    nc.vector.tensor_mul(h[:], t4[:], mbc)
                else:
                    nc.vector.tensor_add(h[:], n[:], t4[:])
                finish(h)

            # ---- software-pipelined tree walk ----
            # Each node is a generator with yields at dependency boundaries
            # (psum-group / activation / elementwise-chain edges).  A driver
            # advances all in-flight nodes round-robin, one stage per tick,
            # so every engine's program-order queue interleaves independent
            # nodes and the per-node serial chains overlap with other nodes'
            # matmul groups instead of head-of-line blocking them.
            hmaps = [{} for _ in range(splits)]
            roots = [None] * splits
            n_nodes = len(order)

            def node_gen(w, t):
                d, j = order[t]
                hmap = hmaps[w]

                def finish(h):
                    hmap[(d, j)] = h

                if d == depth - 1:
                    return emit_leaf(w, t, finish)
                hl = hmap.pop((d + 1, 2 * j))
                hr = hmap.pop((d + 1, 2 * j + 1))
                return emit_internal(w, t, d, hl, hr, finish)

            WIN = int(_os.environ.get("K_WIN", "2"))    # internals in flight
            WINL = int(_os.environ.get("K_WINL", "0"))  # extra leaf lookahead
            active = []                                 # [walk, is_leaf, gen]
            next_i = [0] * splits
            while active or any(next_i[w] < n_nodes for w in range(splits)):
                for w in range(splits):
                    while next_i[w] < n_nodes:
                        if (w > 0 and next_i[w - 1] < n_nodes and
                                next_i[w] + LAG > next_i[w - 1]):
                            break
                        t = next_i[w]
                        d, j = order[t]
                        leaf = d == depth - 1
                        n_int = sum(1 for a in active
                                    if a[0] == w and not a[1])
                        n_all = sum(1 for a in active if a[0] == w)
                        if not leaf and (
                                (d + 1, 2 * j) not in hmaps[w] or
                                (d + 1, 2 * j + 1) not in hmaps[w]):
                            break
                        if leaf:
                            if n_all >= WIN + WINL:
                                break
                        elif n_int >= WIN or n_all >= WIN + WINL:
                            break
                        active.append([w, leaf, node_gen(w, t)])
                        next_i[w] += 1
                for a in list(active):
                    try:
                        next(a[2])
                    except StopIteration:
                        active.remove(a)
            for w in range(splits):
                roots[w] = hmaps[w][(0, 0)]

            # ---- output heads (fp32) ----
            for w in range(splits):
                root = roots[w]
                c0, c1 = w * nsub, (w + 1) * nsub
                pso = psum_tile(w)
                for oi in range(2):
                    for kt in range(HT):
                        nc.tensor.matmul(
                            pso[:, oi, :],
                            lhsT=wout_t[:, kt, oi * O:(oi + 1) * O],
                            rhs=root[:, kt, :],
                            start=(kt == 0),
                            stop=(kt == HT - 1),
                        )
                for oi in range(2):
                    ot = opool.tile([P, nsub], dt.float32, tag="osb", name="osb")
                    if use_bias:
                        act(ot[:], pso[:, oi, :], ACT_F.Identity,
                            bias=bias_t[:, 6 + oi: 7 + oi])
                    else:
                        act(ot[:], pso[:, oi, :], ACT_F.Identity)
                    nc.sync.dma_start(out=out_d[oi, :, c0:c1], in_=ot[:])

    nc.compile()
    _MODULE_CACHE[key] = nc
    return nc


def _to_f32(arr):
    return np.ascontiguousarray(np.asarray(arr, dtype=np.float32))


def _pack_weights(inputs, mode="f32"):
    """Host-side packing of weights into device lhsT layouts."""
    f32 = lambda k: np.asarray(inputs[k], dtype=np.float32)

    def lhsT_h(w):  # [H, H] torch-layout -> [P, HT, H]
        return w.T.reshape(HT, P, w.shape[0]).transpose(1, 0, 2)

    wir_w, wiz_w, win_w = f32("wir_w"), f32("wiz_w"), f32("win_w")
    br = f32("wir_b") + f32("whr_b")
    bz = f32("wiz_b") + f32("whz_b")
    bn = f32("win_b") + f32("whn_b")
    wcat = np.concatenate([wir_w, wiz_w, win_w], axis=0)      # [3H, I]
    bcat = np.concatenate([br, bz, bn])                       # [3H]
    # exact split-K layout: rows = xhi*Whi | 1*bhi | xlo*Whi | 1*blo | xhi*Wlo
    w_hi = _round11(wcat)
    w_lo = wcat - w_hi
    b_hi = _round11(bcat)
    b_lo = bcat - b_hi
    wi = np.concatenate([
        w_hi.T, b_hi[None, :], w_hi.T, b_lo[None, :], w_lo.T], axis=0)

    wms = f32("wms_w")                                        # [2, H, H]
    wsc = f32("w_w").T.reshape(HT, P, 1).transpose(1, 0, 2)   # [P, HT, 1]
    wout = lhsT_h(np.concatenate([f32("mu_w"), f32("lv_w")], axis=0))

    biases = np.zeros((P, 9), dtype=np.float32)
    wms_b = f32("wms_b")                                      # [2, H]
    for k in range(2):
        for mt in range(HT):
            biases[:, 2 * k + mt] = wms_b[k, mt * P:(mt + 1) * P]
    wa_b = f32("wa_b")
    for mt in range(HT):
        biases[:, 4 + mt] = wa_b[mt * P:(mt + 1) * P]
    biases[:, 6] = f32("mu_b")
    biases[:, 7] = f32("lv_b")
    biases[:, 8] = float(np.asarray(inputs["w_b"]).reshape(-1)[0])

    packed = {
        "wi": _to_f32(wi),
        "wsc": _to_f32(wsc),
        "wout": _to_f32(wout),
        "ones1": _to_f32(np.ones((1, P))),
        "ones2": _to_f32(np.ones((2, P))),
        "biases": biases,
    }
    hw_mats = {"whr": f32("whr_w"), "whz": f32("whz_w"), "whn": f32("whn_w"),
               "wa": f32("wa_w"), "wms0": wms[0], "wms1": wms[1]}
    for name, w in hw_mats.items():
        whi = _round11(w)
        wlo = _round11(w - whi)
        packed[name + "hi"] = _to_f32(lhsT_h(whi))
        packed[name + "lo"] = _to_f32(lhsT_h(wlo))

    use_bias = any(
        float(np.abs(np.asarray(inputs[k])).max()) != 0.0
        for k in ("wms_b", "wa_b", "w_b", "mu_b", "lv_b")
    )
    return packed, use_bias


def _pack_percore(targets, mask, use_mask, depth=DEPTH, ncol=NCOL,
                  ncores=NCORES):
    order = _post_order(depth)
    perm = np.array([_gid(d, j) for (d, j) in order])
    nodes = len(order)
    bsz = targets.shape[1]

    tg = np.asarray(targets, dtype=np.float32)[:, :, 0, :]    # [nodes, B, I]
    xall = tg.transpose(2, 0, 1)[:, perm, :]                  # [I, nodes, B]
    x_hi = _round11(xall)
    x_lo = xall - x_hi
    ones = np.ones((1, nodes, bsz), np.float32)
    xaug = np.concatenate([x_hi, ones, x_lo, ones, x_hi], axis=0)  # [KSP,...]

    per_core = []
    for c in range(ncores):
        cols = slice(c * ncol, (c + 1) * ncol)
        pc = {"xT": np.ascontiguousarray(xaug[:, :, cols])}
        if use_mask:
            mpost = np.asarray(mask, dtype=np.float32)[perm]  # [nodes, B]
            pc["maskb"] = np.ascontiguousarray(
                np.broadcast_to(mpost[None, :, cols], (P, nodes, ncol)))
        per_core.append(pc)
    return per_core


def kernel(**inputs):
    import sys
    try:
        import concourse.bass  # noqa: F401
    except ImportError:
        sys.path.insert(0, "/opt/trn_rl_repo")

    try:
        import antenv.axon_hooks  # noqa: F401
    except ImportError:
        # absent in trimmed containers; run_bass_kernel_spmd imports it
        # unconditionally when BASS_TRACE is set — stub the no-hook path
        import types
        _m = types.ModuleType("antenv.axon_hooks")
        _m.get_axon_ntff_profile_hook = lambda: None
        sys.modules["antenv.axon_hooks"] = _m

    from concourse import bass_utils

    packed, use_bias = _pack_weights(inputs)
    use_mask = bool(np.any(np.asarray(inputs["mask"]) != 1.0))
    nc = _build_module(use_bias=use_bias, use_mask=use_mask)
    per_core = _pack_percore(inputs["targets"], inputs["mask"], use_mask)

    in_maps = [{**pc, **packed} for pc in per_core]
    res = bass_utils.run_bass_kernel_spmd(
        nc, in_maps, core_ids=list(range(NCORES)))

    mu = np.empty((B, 1, O), dtype=np.float32)
    lv = np.empty((B, 1, O), dtype=np.float32)
    for c in range(NCORES):
        out = res.results[c]["out"]                          # [2, P, ncol]
        cols = slice(c * NCOL, (c + 1) * NCOL)
        mu[cols, 0, :] = out[0].T
        lv[cols, 0, :] = out[1].T
    return mu, lv
